# revision 1
# baseline (speedup 1.0000x reference)
"""Contrastive loss (InfoNCE, diagonal labels) Trainium2 kernel.

loss = -mean_i log_softmax(E_n @ E_n.T / T)[i, i],  E_n = L2-normalized rows.

Rewritten per-row as  loss_i = log( sum_j exp((s_ij - s_ii) / T) )  which is
exact (s_ii is the row max since rows are unit vectors) and numerically stable:
the diagonal term of the sum is exactly 1.

Sharding: row-parallel over 8 cores. Each core receives the FULL embeddings
(for the key side) plus its own 2048-row slice, computes its [2048, 16384]
logits block tile-by-tile (never materialized), and outputs its 2048 per-row
losses; the host takes the mean. No collectives needed.

Per-core dataflow:
  prologue: normalize rows in fp32, cast to bf16, PE-transpose to [d, rows]
  main:     PE bf16 matmuls (K=256 via 2 PSUM-accumulated chunks) fill
            [128, 2048] PSUM tiles; ScalarE reads PSUM directly doing
            exp(scale*x + bias_i) with fused accum_out row-sums, so the
            N^2 = 268M exponentials never touch the vector engine.
"""

import os
import sys

sys.path.insert(0, "/opt/trn_rl_repo")

from contextlib import ExitStack

import numpy as np

import concourse.bass as bass
import concourse.tile as tile
from concourse import bacc, masks, mybir
from concourse.bass_utils import run_bass_kernel_spmd

# The act-table insertion pass greedily picks the first table-set containing
# each activation function, so a kernel alternating Ln and Exp thrashes
# between `natural_log` and `exp_and_others` (~2.7us per ACT_TABLE_LOAD, one
# per switch). Both functions live together in `natural_log_exp_and_others`;
# hide them from every other set (positions preserved — act_func_set_id is
# positional) so the pass serves Ln and Exp from the combined set with a
# single load.
_orig_get_act_tables = bacc.get_activation_tables


def _combined_exp_ln_tables(arch):
    tabs = _orig_get_act_tables(arch)
    both = mybir.ActivationFunctionType.Exp, mybir.ActivationFunctionType.Ln
    out = {}
    for name, fns in tabs.items():
        if name != "natural_log_exp_and_others" and all(f in fns for f in both):
            name_keep = False
        else:
            name_keep = name == "natural_log_exp_and_others"
        if not name_keep:
            fns = {f for f in fns if f not in both}
        out[name] = fns
    return out


bacc.get_activation_tables = _combined_exp_ln_tables

N = 16384  # total rows
D = 256  # embedding dim
P = 128  # partitions
CORES = 8
R = N // CORES  # rows per core = 2048
GF = N // P  # 128 row-groups total
GR = R // P  # 16 row-groups per core
CG = 16  # groups per prologue chunk (16*128 = 2048 rows, 2MB fp32)
NCH_F = GF // CG  # 16 full-side chunks
NCH_R = GR // CG  # 2 row-side chunks
JB = 4  # PSUM banks per ScalarE call -> free dim 2048
NJ = 512  # matmul free dim (one PSUM bank, fp32)
JGRP = N // (JB * NJ)  # 8 j-groups per row-block
TEMP = 0.07
SCALE = float(1.0 / TEMP)
PACE_MM = 0  # extra matmuls per PSUM tile to keep the PE clock ramped

f32 = mybir.dt.float32
bf16 = mybir.dt.bfloat16
MULT = mybir.AluOpType.mult
EXP = mybir.ActivationFunctionType.Exp
LN = mybir.ActivationFunctionType.Ln
SQUARE = mybir.ActivationFunctionType.Square
AXX = mybir.AxisListType.X


def _norm_chunk(nc, pools, t, n_u, src_g, dstT, ident, ss_pool_tag, ssb=None, act_ss=False):
    """Normalize chunk t (n_u row-groups): load raw fp32, compute per-row
    1/||x||, scale+cast to bf16, PE-transpose into dstT[kc][t] ([d, row] bf16).

    The DMA landing tile `raw` has exactly two readers (whole-chunk square and
    whole-chunk scale) — HW DMA descriptors only support a few sync waits, so
    the recycled slot's WAR dependencies must stay tiny.

    If ssb is given (rows side), also writes sum_d(bf16 operand ^2) into
    ssb[:, g] for each group g — the exact value the matmul diagonal produces,
    used as the softmax-shift bias."""
    loads, normp, psum, dumps, small = pools
    raw = loads.tile([P, n_u, D], f32, tag="raw")
    nc.sync.dma_start(raw[:], src_g[:, t * CG : t * CG + n_u, :])

    ss = small.tile([P, n_u], f32, tag=ss_pool_tag + "_ss", bufs=4)
    if act_ss:
        # head chunks: ACT is idle before the main loop starts, and Square
        # lives in the same table set as Exp — do sum-of-squares there to
        # shorten the serial DVE chain in front of the first matmuls
        sqd = dumps.tile([P, D], f32, tag="sqd")
        for u in range(n_u):
            nc.scalar.activation(
                sqd[:], raw[:, u, :], SQUARE, accum_out=ss[:, u : u + 1]
            )
    else:
        # fused square+row-sum per group: shorter DVE chain latency than a
        # whole-chunk square followed by a whole-chunk reduce
        sqd0 = dumps.tile([P, D], f32, tag="sqd0")
        for u in range(n_u):
            nc.vector.scalar_tensor_tensor(
                out=sqd0[:],
                in0=raw[:, u, :],
                scalar=1.0,
                in1=raw[:, u, :],
                op0=MULT,
                op1=MULT,
                accum_out=ss[:, u : u + 1],
            )

    # rinv = ss^-0.5 = exp(-0.5 * ln(ss)); Ln+Exp share one ACT table set
    lnb = small.tile([P, n_u], f32, tag=ss_pool_tag + "_ln", bufs=4)
    rinv = small.tile([P, n_u], f32, tag=ss_pool_tag + "_ri", bufs=4)
    nc.scalar.activation(lnb[:], ss[:], LN)
    nc.scalar.activation(rinv[:], lnb[:], EXP, scale=-0.5)

    nbf = normp.tile([P, n_u, D], bf16, tag="nbf")
    for u in range(n_u):
        nc.vector.tensor_scalar_mul(nbf[:, u, :], raw[:, u, :], rinv[:, u : u + 1])
    if ssb is not None:
        sqd2 = dumps.tile([P, D], f32, tag="sqd2")
        for u in range(n_u):
            if act_ss:
                nc.scalar.activation(
                    sqd2[:],
                    nbf[:, u, :],
                    SQUARE,
                    accum_out=ssb[:, t * CG + u : t * CG + u + 1],
                )
            else:
                nc.vector.scalar_tensor_tensor(
                    out=sqd2[:],
                    in0=nbf[:, u, :],
                    scalar=1.0,
                    in1=nbf[:, u, :],
                    op0=MULT,
                    op1=MULT,
                    accum_out=ssb[:, t * CG + u : t * CG + u + 1],
                )
    # PE transpose each [128, 128] block; pack per-kc so one DVE copy moves
    # all n_u blocks of a kc to SBUF. Shares the "ps" PSUM tag with the main
    # loop's tiles (2 x 4-bank slots).
    pst = psum.tile([P, 2 * n_u * P], bf16, tag="ps")
    for kc in range(2):
        for u in range(n_u):
            blk = (kc * n_u + u) * P
            nc.tensor.transpose(
                pst[:, blk : blk + P], nbf[:, u, kc * P : (kc + 1) * P], ident[:]
            )
    for kc in range(2):
        nc.vector.tensor_copy(dstT[kc][t][:], pst[:, kc * n_u * P : (kc + 1) * n_u * P])


def build_program():
    nc = bacc.Bacc("TRN2", target_bir_lowering=False, debug=False, num_devices=CORES)
    emb = nc.dram_tensor("embeddings", [N, D], f32, kind="ExternalInput").ap()
    emb_rows = nc.dram_tensor("emb_rows", [R, D], f32, kind="ExternalInput").ap()
    out = nc.dram_tensor("out_rows", [R], f32, kind="ExternalOutput").ap()

    with tile.TileContext(nc) as tc:
        with ExitStack() as ctx:
            persist = ctx.enter_context(tc.tile_pool(name="persist", bufs=1))
            loads = ctx.enter_context(tc.tile_pool(name="loads", bufs=3))
            normp = ctx.enter_context(tc.tile_pool(name="normp", bufs=4))
            psum = ctx.enter_context(
                tc.tile_pool(name="psum", bufs=2, space=bass.MemorySpace.PSUM)
            )
            dumps = ctx.enter_context(tc.tile_pool(name="dumps", bufs=2))
            small = ctx.enter_context(tc.tile_pool(name="small", bufs=1))
            pools = (loads, normp, psum, dumps, small)

            ident = persist.tile([P, P], bf16, name="ident")
            masks.make_identity(nc, ident[:])

            # keys/queries, transposed+normalized, chunked so the scheduler can
            # overlap the main loop with later prologue chunks
            embT = [
                [persist.tile([P, CG * P], bf16, name=f"embT_{kc}_{t}") for t in range(NCH_F)]
                for kc in range(2)
            ]
            rowsT = [
                [persist.tile([P, CG * P], bf16, name=f"rowsT_{kc}_{t}") for t in range(NCH_R)]
                for kc in range(2)
            ]
            ssb = persist.tile([P, GR], f32, name="ssb")
            sp_all = persist.tile([P, GR * JGRP], f32, name="sp_all")
            bias = persist.tile([P, GR], f32, name="bias")
            s_col = persist.tile([P, GR], f32, name="s_col")
            lout = persist.tile([P, GR], f32, name="lout")

            rows_g = emb_rows.rearrange("(u p) d -> p u d", p=P)
            emb_g = emb.rearrange("(u p) d -> p u d", p=P)

            # K chunk 0 first: its DVE chain (stt squares + scale) interleaves
            # under the rows side's ACT-heavy chain, shortening the head
            _norm_chunk(nc, pools, 0, CG, emb_g, embT, ident, "f")
            for t in range(NCH_R):
                _norm_chunk(nc, pools, t, CG, rows_g, rowsT, ident, "r", ssb=ssb, act_ss=True)
            nc.vector.tensor_scalar_mul(bias[:], ssb[:], -SCALE)

            # main: OUTER loop over j-groups so each one only needs the two
            # embT chunks prepared just before it — the key-side prologue
            # streams concurrently with main compute instead of serializing
            # ~150us in front of it. Inner loop over the 16 own-row groups.
            #
            # The TensorE clock only ramps to 2.4 GHz after ~3us of
            # *continuous* execution; any idle resets it to 1.2 GHz. ScalarE's
            # exp (the steady bottleneck) is within a few percent of PE's
            # matmul time per PSUM tile, so PACE_MM extra matmuls per tile
            # keep PE strictly the busiest engine (their output is reset by
            # the first real matmul's start=True).
            for jj in range(JGRP):
                for g in range(GR):
                    if g == 8 and jj + 1 < JGRP:
                        _norm_chunk(nc, pools, jj + 1, CG, emb_g, embT, ident, "f")
                    rt = g // CG
                    ro = (g % CG) * P
                    pm = psum.tile([P, JB * NJ], f32, tag="ps")
                    for _ in range(PACE_MM):
                        nc.tensor.matmul(
                            pm[:, 0:NJ],
                            rowsT[0][rt][:, ro : ro + P],
                            embT[0][jj][:, 0:NJ],
                            start=True,
                            stop=True,
                        )
                    for jb in range(JB):
                        jc = jj * JB + jb  # 512-col chunk index
                        ft, fo = jc // (CG * P // NJ), (jc % (CG * P // NJ)) * NJ
                        for kc in range(2):
                            nc.tensor.matmul(
                                pm[:, jb * NJ : (jb + 1) * NJ],
                                rowsT[kc][rt][:, ro : ro + P],
                                embT[kc][ft][:, fo : fo + NJ],
                                start=(kc == 0),
                                stop=(kc == 1),
                            )
                    dmp = dumps.tile([P, JB * NJ], f32, tag="dmp")
                    nc.scalar.activation(
                        dmp[:],
                        pm[:],
                        EXP,
                        bias=bias[:, g : g + 1],
                        scale=SCALE,
                        accum_out=sp_all[:, g * JGRP + jj : g * JGRP + jj + 1],
                    )
            for g in range(GR):
                nc.vector.reduce_sum(
                    s_col[:, g : g + 1],
                    sp_all[:, g * JGRP : (g + 1) * JGRP],
                    axis=AXX,
                )
            nc.scalar.activation(lout[:], s_col[:], LN)
            nc.sync.dma_start(out.rearrange("(u p) -> p u", p=P), lout[:])

    nc.compile()
    return nc


def run_cores(embeddings: np.ndarray, trace: bool = False):
    nc = build_program()
    in_maps = [
        {
            "embeddings": embeddings,
            "emb_rows": np.ascontiguousarray(embeddings[c * R : (c + 1) * R]),
        }
        for c in range(CORES)
    ]
    return run_bass_kernel_spmd(nc, in_maps, list(range(CORES)), trace=trace)


def kernel(embeddings: np.ndarray) -> np.ndarray:
    embeddings = np.ascontiguousarray(np.asarray(embeddings, dtype=np.float32))
    assert embeddings.shape == (N, D)
    res = run_cores(embeddings)
    vals = np.concatenate([res.results[c]["out_rows"] for c in range(CORES)])
    return np.float32(vals.mean())



# revision 14
# speedup vs baseline: 1.4461x; 1.4461x over previous
"""Contrastive loss (InfoNCE, diagonal labels) Trainium2 kernel.

loss = -mean_i log_softmax(E_n @ E_n.T / T)[i, i],  E_n = L2-normalized rows.

Key ideas vs the row-parallel baseline:

1. Symmetry: s_ij = s_ji, so the N x N exp() matrix only needs its "upper
   half" computed. Work is assigned at 2048-row block granularity: core c
   computes its diagonal block, its blocks against cols of blocks c+1..c+3
   (mod 8), and half of the pair block c+4 (split into the two anti-diagonal
   quadrants so both cores of a pair run an identical program). Row sums for
   the mirrored (never-computed) blocks are recovered as COLUMN sums of the
   computed exp tiles via a cheap ones-vector PE matmul; the host combines
   row/col partials, applies per-row softmax shifts, and takes log+mean.
   ScalarE exp work (the hard bottleneck: ~0.83 ns/elem, no dtype speedup)
   drops to 36/64 of the full matrix.

2. fp8 DoubleRow matmuls: both operands quantized to fp8e4 (after fp32
   normalization), K=256 contracted in ONE matmul (two 128-row groups packed
   per partition) at 0.5 cycles/row -> 4x less PE time than bf16 K-chunked.
   Relative-error budget is 2e-2; fp8 dot noise is ~0.3% on exp arguments.

3. Diagonal exactness: the softmax shift uses ssb_i = sum_d fp8(v_d)^2 --
   exactly what the PE produces for s_ii -- so the diagonal exp is 1 and the
   row sum is 1 + small terms (no cancellation). Off-diagonal blocks are
   exp'd UNSHIFTED (args in [-5.7, 5.7], safe in bf16/fp32); the host applies
   exp(-scale*ssb_i) when merging, keeping everything cancellation-free.

Per-core dataflow:
  prologue: 5 col-chunks of 2048 rows stream in; normalize fp32, cast fp8,
  PE-transpose to [d, row] with both K-halves packed per chunk tile.
  main: 80 PSUM tiles ([128,2048] for blocks 0-3, [128,1024] for block 4):
  fp8 DR matmuls fill PSUM; one ScalarE exp per tile (PSUM->SBUF bf16) with
  fused accum row-sums; ones-matmul column sums land in the just-freed PSUM
  bank and DVE-accumulate into SBUF; host merges all partials in f64.
"""

import os
import sys

sys.path.insert(0, "/opt/trn_rl_repo")

from contextlib import ExitStack

import numpy as np

import concourse.bass as bass
import concourse.tile as tile
from concourse import bacc, masks, mybir
from concourse.bass_utils import run_bass_kernel_spmd

# The act-table insertion pass greedily picks the first table-set containing
# each activation function, so a kernel alternating Ln and Exp thrashes
# between `natural_log` and `exp_and_others` (~2.7us per ACT_TABLE_LOAD, one
# per switch). Both functions live together in `natural_log_exp_and_others`;
# hide them from every other set (positions preserved — act_func_set_id is
# positional) so the pass serves Ln and Exp from the combined set with a
# single load.
_orig_get_act_tables = bacc.get_activation_tables


def _combined_exp_ln_tables(arch):
    tabs = _orig_get_act_tables(arch)
    both = mybir.ActivationFunctionType.Exp, mybir.ActivationFunctionType.Ln
    out = {}
    for name, fns in tabs.items():
        if name != "natural_log_exp_and_others" and all(f in fns for f in both):
            name_keep = False
        else:
            name_keep = name == "natural_log_exp_and_others"
        if not name_keep:
            fns = {f for f in fns if f not in both}
        out[name] = fns
    return out


bacc.get_activation_tables = _combined_exp_ln_tables

N = 16384  # total rows
D = 256  # embedding dim
P = 128  # partitions
CORES = 8
R = N // CORES  # rows per block = 2048
CG = 16  # row-groups per chunk (16*128 = 2048 rows)
NCH = 5  # col-chunks per core: diag + 3 full pairs + half-split pair
GR = 16  # own row-groups per core
NJ = 512  # matmul free dim (one PSUM bank fp32)
TEMP = 0.07
SCALE = float(1.0 / TEMP)

f32 = mybir.dt.float32
bf16 = mybir.dt.bfloat16
fp8 = mybir.dt.float8e4
MULT = mybir.AluOpType.mult
ADD = mybir.AluOpType.add
EXP = mybir.ActivationFunctionType.Exp
LN = mybir.ActivationFunctionType.Ln
DR = mybir.MatmulPerfMode.DoubleRow


def build_program():
    nc = bacc.Bacc("TRN2", target_bir_lowering=False, debug=False, num_devices=CORES)
    embp = nc.dram_tensor("embp", [NCH * R, D], f32, kind="ExternalInput").ap()
    sp_out = nc.dram_tensor("sp_out", [P, 80], f32, kind="ExternalOutput").ap()
    ssb_out = nc.dram_tensor("ssb_out", [P, GR], f32, kind="ExternalOutput").ap()
    self_out = nc.dram_tensor("self_out", [P, GR], f32, kind="ExternalOutput").ap()
    col_out = nc.dram_tensor("col_out", [P, 4 * NJ], f32, kind="ExternalOutput").ap()

    emb_g = embp.rearrange("(t u p) d -> p (t u) d", u=CG, p=P)

    with tile.TileContext(nc) as tc:
        with ExitStack() as ctx:
            persist = ctx.enter_context(tc.tile_pool(name="persist", bufs=1))
            loads = ctx.enter_context(tc.tile_pool(name="loads", bufs=2))
            prep = ctx.enter_context(tc.tile_pool(name="prep", bufs=2))
            psum = ctx.enter_context(
                tc.tile_pool(name="psum", bufs=1, space=bass.MemorySpace.PSUM)
            )
            exps = ctx.enter_context(tc.tile_pool(name="exps", bufs=2))
            dumps = ctx.enter_context(tc.tile_pool(name="dumps", bufs=2))
            small = ctx.enter_context(tc.tile_pool(name="small", bufs=4))

            ident = persist.tile([P, P], fp8, name="ident")
            masks.make_identity(nc, ident[:])
            identb = persist.tile([P, P], bf16, name="identb")
            masks.make_identity(nc, identb[:])
            ones = persist.tile([P, 1], bf16, name="ones")
            nc.gpsimd.memset(ones[:], 1.0)

            # transposed fp8 chunks: [d-partition, kc, row]; kc packs the two
            # 128-wide halves of the embedding dim for DoubleRow matmuls
            embT = [
                persist.tile([P, 2, CG * P], fp8, name=f"embT_{t}") for t in range(NCH)
            ]
            ssb = persist.tile([P, GR], f32, name="ssb")
            bias = persist.tile([P, GR], f32, name="bias")
            sp = persist.tile([P, 80], f32, name="sp")
            selfT = persist.tile([P, GR], f32, name="selfT")
            colstage = persist.tile([P, 4 * NJ], f32, name="colstage")

            def prep_chunk(t, with_ssb=False):
                """DMA chunk t, normalize fp32, write fp8 nbf [P, CG, D]."""
                raw = loads.tile([P, CG, D], f32, tag="raw")
                nc.sync.dma_start(raw[:], emb_g[:, t * CG : (t + 1) * CG, :])
                ss = small.tile([P, CG], f32, tag="ss")
                sqd = dumps.tile([P, D], f32, tag="sqd")
                for u in range(CG):
                    nc.vector.scalar_tensor_tensor(
                        out=sqd[:],
                        in0=raw[:, u, :],
                        scalar=1.0,
                        in1=raw[:, u, :],
                        op0=MULT,
                        op1=MULT,
                        accum_out=ss[:, u : u + 1],
                    )
                # rinv = ss^-0.5 = exp(-0.5*ln(ss)); Ln+Exp share a table set
                lnb = small.tile([P, CG], f32, tag="ln")
                rinv = small.tile([P, CG], f32, tag="ri")
                nc.scalar.activation(lnb[:], ss[:], LN)
                nc.scalar.activation(rinv[:], lnb[:], EXP, scale=-0.5)
                nbf = prep.tile([P, CG, D], fp8, tag="nbf")
                for u in range(CG):
                    nc.vector.tensor_scalar_mul(
                        nbf[:, u, :], raw[:, u, :], rinv[:, u : u + 1]
                    )
                if with_ssb:
                    # ssb = sum_d fp8(v)^2: exactly the PE's diagonal value
                    sqd2 = dumps.tile([P, D], f32, tag="sqd2")
                    for u in range(CG):
                        nc.vector.scalar_tensor_tensor(
                            out=sqd2[:],
                            in0=nbf[:, u, :],
                            scalar=1.0,
                            in1=nbf[:, u, :],
                            op0=MULT,
                            op1=MULT,
                            accum_out=ssb[:, u : u + 1],
                        )
                return nbf

            def prep_transpose(t, nbf, kc, slot):
                """PE-transpose kc-half of chunk t into embT[t][:, kc, :],
                staging through the (currently free) front of a PSUM slot.
                FP8 transpose hardware writes outputs at element step 2."""
                pst = psum.tile([P, 2 * CG * P], fp8, tag=slot)
                for u in range(CG):
                    nc.tensor.transpose(
                        pst[:, u * 2 * P : (u + 1) * 2 * P : 2],
                        nbf[:, u, kc * P : (kc + 1) * P],
                        ident[:],
                    )
                nc.vector.tensor_copy(embT[t][:, kc, :], pst[:, 0::2])

            nbf0 = prep_chunk(0, with_ssb=True)
            nc.vector.tensor_scalar_mul(bias[:], ssb[:], -SCALE)
            prep_transpose(0, nbf0, 0, "psA")
            prep_transpose(0, nbf0, 1, "psB")

            nbf_next = None
            for blk in range(NCH):
                W = R if blk < 4 else R // 2  # tile width (cols)
                NB = W // NJ  # PSUM banks / jb slices per tile
                for g in range(GR):
                    # stream the next chunk's prep under this block's compute
                    if blk < 4:
                        if g == 1:
                            nbf_next = prep_chunk(blk + 1)
                        elif g == 8:
                            prep_transpose(blk + 1, nbf_next, 0, "psA")
                        elif g == 9:
                            prep_transpose(blk + 1, nbf_next, 1, "psB")

                    slot = "psA" if g % 2 == 0 else "psB"
                    off = 0 if (blk < 4 or g < 8) else R // 2
                    pm = psum.tile([P, W], f32, tag=slot)
                    for jb in range(NB):
                        nc.tensor.matmul(
                            pm[:, jb * NJ : (jb + 1) * NJ],
                            embT[0][:, :, g * P : (g + 1) * P],
                            embT[blk][:, :, off + jb * NJ : off + (jb + 1) * NJ],
                            start=True,
                            stop=True,
                            perf_mode=DR,
                        )
                    if blk == 0:
                        # f32 out: the self terms get subtracted on the host
                        # and must carry full precision
                        expM = exps.tile([P, W], f32, tag="expd")
                        nc.scalar.activation(
                            expM[:],
                            pm[:],
                            EXP,
                            bias=bias[:, g : g + 1],
                            scale=SCALE,
                            accum_out=sp[:, g : g + 1],
                        )
                        # extract exp'd self terms: PE's fp8 s_ii lands a few
                        # e-5 below the DVE's ssb, so exp(scale*(s_ii-ssb_i))
                        # is NOT exactly 1 -- the host subtracts the true
                        # device value instead of assuming 1
                        sdump = dumps.tile([P, P], f32, tag="sdump")
                        nc.vector.scalar_tensor_tensor(
                            out=sdump[:],
                            in0=expM[:, g * P : (g + 1) * P],
                            scalar=1.0,
                            in1=identb[:],
                            op0=MULT,
                            op1=MULT,
                            accum_out=selfT[:, g : g + 1],
                        )
                    else:
                        expM = exps.tile([P, W], bf16, tag="exp")
                        nc.scalar.activation(
                            expM[:],
                            pm[:],
                            EXP,
                            scale=SCALE,
                            accum_out=sp[:, blk * GR + g : blk * GR + g + 1],
                        )
                        # column sums -> just-freed bank0 of this slot, then
                        # DVE-accumulate into SBUF (unused partitions carry
                        # garbage that the host never reads)
                        colp = psum.tile([P, NJ], f32, tag=slot)
                        pbase = 0 if (blk < 4 or g < 8) else 64
                        for jb in range(NB):
                            pb = pbase + 32 * jb
                            nc.tensor.matmul(
                                colp[pb : pb + 1, :],
                                ones[:],
                                expM[:, jb * NJ : (jb + 1) * NJ],
                                start=True,
                                stop=True,
                                tile_position=(0, pb),
                            )
                        cs = colstage[:, (blk - 1) * NJ : blk * NJ]
                        if blk < 4:
                            rng = slice(0, P)
                            first = g == 0
                        else:
                            rng = slice(0, 64) if g < 8 else slice(64, P)
                            first = g in (0, 8)
                        if first:
                            nc.vector.tensor_copy(cs[rng, :], colp[rng, :])
                        else:
                            nc.vector.scalar_tensor_tensor(
                                out=cs[rng, :],
                                in0=colp[rng, :],
                                scalar=1.0,
                                in1=cs[rng, :],
                                op0=MULT,
                                op1=ADD,
                            )

            nc.sync.dma_start(sp_out, sp[:])
            nc.sync.dma_start(ssb_out, ssb[:])
            nc.sync.dma_start(self_out, selfT[:])
            nc.sync.dma_start(col_out, colstage[:])

    nc.compile()
    return nc


def _host_inputs(embeddings: np.ndarray):
    """Per-core concatenated col-chunk inputs (the sharding schedule)."""
    blocks = [embeddings[b * R : (b + 1) * R] for b in range(CORES)]
    in_maps = []
    for c in range(CORES):
        parts = [blocks[(c + k) % 8] for k in range(4)]
        p4 = blocks[(c + 4) % 8]
        if c >= 4:
            # anti-diagonal quadrant split: this core's tiles for groups 0-7
            # must see the partner's SECOND half first
            p4 = np.concatenate([p4[R // 2 :], p4[: R // 2]])
        parts.append(p4)
        in_maps.append({"embp": np.ascontiguousarray(np.concatenate(parts))})
    return in_maps


def run_cores(embeddings: np.ndarray, trace: bool = False):
    nc = build_program()
    in_maps = _host_inputs(embeddings)
    return run_bass_kernel_spmd(nc, in_maps, list(range(CORES)), trace=trace)


def _combine(results) -> np.float32:
    """Merge per-core row/col partials into the loss (f64 on host)."""
    S_diag = np.zeros(N)  # shifted diagonal-block row sums
    self_all = np.zeros(N)  # device-computed exp'd self terms
    U = np.zeros(N)  # unshifted off-diagonal sums
    ssb_all = np.zeros(N)
    for c in range(CORES):
        sp = np.asarray(results[c]["sp_out"], dtype=np.float64)  # [P, 80]
        ssb = np.asarray(results[c]["ssb_out"], dtype=np.float64)  # [P, GR]
        col = np.asarray(results[c]["col_out"], dtype=np.float64)  # [P, 2048]
        rows = slice(c * R, (c + 1) * R)
        # local row r = g*128 + p  ->  [g, p] raveled
        ssb_all[rows] = ssb.T.ravel()
        self_all[rows] = np.asarray(
            results[c]["self_out"], dtype=np.float64
        ).T.ravel()
        S_diag[rows] = sp[:, 0:GR].T.ravel()
        U[c * R : (c + 1) * R] += sp[:, GR:80].reshape(P, 4, GR).sum(axis=1).T.ravel()
        for k in range(1, 5):
            vals = col[0:P:32, (k - 1) * NJ : k * NJ].ravel()  # pos = jb*512+j
            b = (c + k) % 8
            if k == 4 and c >= 4:
                vals = np.roll(vals, R // 2)
            U[b * R : (b + 1) * R] += vals
    # The reference similarity uses exactly-unit rows, while the device works
    # on fp8 rows with ||q_i||^2 = ssb_i != 1: subtract the device's own
    # exp'd self term, rescale the diag-block remainder from the ssb_i shift
    # to the true unit-norm shift, and shift the unshifted off-block sums by
    # the constant e^-scale. The true self softmax term is exactly 1.
    S = 1.0 + (S_diag - self_all) * np.exp(SCALE * (ssb_all - 1.0)) + np.exp(-SCALE) * U
    return np.float32(np.mean(np.log(S)))


def kernel(embeddings: np.ndarray) -> np.ndarray:
    embeddings = np.ascontiguousarray(np.asarray(embeddings, dtype=np.float32))
    assert embeddings.shape == (N, D)
    res = run_cores(embeddings)
    return _combine(res.results)
